# revision 7
# baseline (speedup 1.0000x reference)
"""Chamfer distance (CDLoss) Trainium2 kernel.

Problem: prediction [4, 8192, 3], ground_truth [4, 8192, 3] (fp32).
For each batch: d2[n,m] = max(||p_n||^2 + ||g_m||^2 - 2 p.g, 0);
out[b] = sum_n min_m d2 / N + sum_m min_n d2 / M.

Strategy (8 NeuronCores): core c handles (batch = c//2, row-half = c%2),
i.e. a 4096 x 8192 slab of the distance matrix.

Device kernel per core:
  - Augmented-coordinate trick: ap[5, 4096] = [px, py, pz, ||p||^2, 1],
    ag[5, 8192] = [-2gx, -2gy, -2gz, 1, ||g||^2] so a single K=5 fp32
    matmul tile emits squared distances directly into PSUM.
  - Tiles [128 rows x 2048 cols] (4 matmuls of N=512 into 4 PSUM banks).
  - One tensor_tensor_reduce per tile: copies PSUM -> SBUF (bf16) and
    min-reduces each row into a per-(rowblock, colgroup) partial (fp32,
    exact).
  - One bf16 tensor_tensor(min) per tile: running column-min buffer
    [128, 8192] (bf16 is exact-monotone: min of rounded = rounded min).
Host: final tiny reductions (min over 128 partitions / 4 groups, relu,
sums) in numpy.
"""

import numpy as np

_B = 4
_N = 8192  # points per cloud
_HALF = _N // 2  # rows per core
_RB = _HALF // 128  # 32 row blocks
_GW = 512  # column group width (one PSUM bank)
_G = _N // _GW  # 16 column groups
_NCORES = 8

_CACHED_NC = None
_RUNNERS = {}


def _build_nc():
    import concourse.bacc as bacc
    import concourse.tile as tile
    from concourse import mybir

    f32 = mybir.dt.float32
    bf16 = mybir.dt.bfloat16

    nc = bacc.Bacc("TRN2", target_bir_lowering=False, debug=False)

    ap_d = nc.dram_tensor("ap", [5, _HALF], f32, kind="ExternalInput")
    ag_d = nc.dram_tensor("ag", [5, _N], f32, kind="ExternalInput")
    rowparts_d = nc.dram_tensor(
        "rowparts", [128, _RB * _G], f32, kind="ExternalOutput"
    )
    colmin_d = nc.dram_tensor("colmin", [128, _N], bf16, kind="ExternalOutput")

    BIG = 1.0e38

    with tile.TileContext(nc) as tc:
        with (
            tc.tile_pool(name="singles", bufs=1) as singles,
            tc.tile_pool(name="spool", bufs=6) as spool,
            tc.tile_pool(name="psum", bufs=8, space="PSUM") as pp,
        ):
            ap_s = singles.tile([5, _HALF], f32)
            nc.sync.dma_start(out=ap_s[:], in_=ap_d[:])
            ag_s = singles.tile([5, _N], f32)
            nc.sync.dma_start(out=ag_s[:], in_=ag_d[:])

            colmin_s = singles.tile([128, _N], bf16)
            nc.vector.memset(colmin_s[:], BIG)
            rowparts_s = singles.tile([128, _RB * _G], f32)

            for rb in range(_RB):
                lhsT = ap_s[:, rb * 128 : (rb + 1) * 128]
                for g in range(_G):
                    t = pp.tile([128, _GW], f32, tag="t")
                    nc.tensor.matmul(
                        t[:],
                        lhsT,
                        ag_s[:, g * _GW : (g + 1) * _GW],
                        start=True,
                        stop=True,
                    )
                    idx = rb * _G + g
                    # exact fp32 row minima for this tile (DVE, PSUM src)
                    nc.vector.tensor_reduce(
                        rowparts_s[:, idx : idx + 1],
                        t[:],
                        axis=mybir.AxisListType.X,
                        op=mybir.AluOpType.min,
                    )
                    # PSUM -> SBUF exit on ScalarE, cast to bf16
                    s = spool.tile([128, _GW], bf16, tag="s")
                    nc.scalar.copy(s[:], t[:])
                    # running column minima (bf16, DVE 2x mode)
                    cslice = colmin_s[:, g * _GW : (g + 1) * _GW]
                    nc.vector.tensor_tensor(
                        cslice, cslice, s[:], op=mybir.AluOpType.min
                    )

            nc.sync.dma_start(out=rowparts_d[:], in_=rowparts_s[:])
            nc.sync.dma_start(out=colmin_d[:], in_=colmin_s[:])

    nc.compile()
    return nc


def _get_nc():
    global _CACHED_NC
    if _CACHED_NC is None:
        _CACHED_NC = _build_nc()
    return _CACHED_NC


def _prep_core_inputs(prediction, ground_truth):
    """Build per-core augmented matrices (host-side, fp32)."""
    in_maps = []
    for c in range(_NCORES):
        b, h = divmod(c, 2)
        p = np.asarray(prediction[b, h * _HALF : (h + 1) * _HALF], dtype=np.float32)
        g = np.asarray(ground_truth[b], dtype=np.float32)
        ap = np.empty((5, _HALF), dtype=np.float32)
        ap[0:3] = p.T
        ap[3] = (p * p).sum(axis=1, dtype=np.float32)
        ap[4] = 1.0
        ag = np.empty((5, _N), dtype=np.float32)
        ag[0:3] = (-2.0 * g).T
        ag[3] = 1.0
        ag[4] = (g * g).sum(axis=1, dtype=np.float32)
        in_maps.append({"ap": ap, "ag": ag})
    return in_maps


def _make_runner(nc, n_cores):
    """Build a cached jitted SPMD executor for `nc` (axon/PJRT path).

    Mirrors concourse.bass2jax.run_bass_via_pjrt but caches the jitted
    callable so repeat calls don't re-trace/re-compile.
    """
    import jax
    import numpy as _np
    from jax.sharding import Mesh, PartitionSpec
    from jax.experimental.shard_map import shard_map
    from concourse import mybir
    from concourse.bass2jax import (
        _bass_exec_p,
        install_neuronx_cc_hook,
        partition_id_tensor,
    )

    install_neuronx_cc_hook()

    partition_name = (
        nc.partition_id_tensor.name if nc.partition_id_tensor else None
    )
    in_names, out_names, out_avals, zero_shapes = [], [], [], []
    for alloc in nc.m.functions[0].allocations:
        if not isinstance(alloc, mybir.MemoryLocationSet):
            continue
        name = alloc.memorylocations[0].name
        if alloc.kind == "ExternalInput":
            if name == partition_name:
                continue
            in_names.append(name)
        elif alloc.kind == "ExternalOutput":
            shape = tuple(alloc.tensor_shape)
            dtype = mybir.dt.np(alloc.dtype)
            out_names.append(name)
            out_avals.append(jax.core.ShapedArray(shape, dtype))
            zero_shapes.append((shape, dtype))
    n_params = len(in_names)
    n_outs = len(out_names)
    all_names = in_names + out_names
    if partition_name is not None:
        all_names = all_names + [partition_name]
    donate = tuple(range(n_params, n_params + n_outs))

    def _body(*args):
        operands = list(args)
        if partition_name is not None:
            operands.append(partition_id_tensor())
        outs = _bass_exec_p.bind(
            *operands,
            out_avals=tuple(out_avals),
            in_names=tuple(all_names),
            out_names=tuple(out_names),
            lowering_input_output_aliases=(),
            sim_require_finite=True,
            sim_require_nnan=True,
            nc=nc,
        )
        return tuple(outs)

    devices = jax.devices()[:n_cores]
    mesh = Mesh(_np.asarray(devices), ("core",))
    sharded = jax.jit(
        shard_map(
            _body,
            mesh=mesh,
            in_specs=(PartitionSpec("core"),) * (n_params + n_outs),
            out_specs=(PartitionSpec("core"),) * n_outs,
            check_rep=False,
        ),
        donate_argnums=donate,
        keep_unused=True,
    )

    def run(in_maps):
        concat_in = [
            _np.concatenate([m[name] for m in in_maps], axis=0)
            for name in in_names
        ]
        concat_zeros = [
            _np.zeros((n_cores * s[0], *s[1:]), d) for (s, d) in zero_shapes
        ]
        out_arrs = sharded(*concat_in, *concat_zeros)
        return [
            {
                name: _np.asarray(out_arrs[i]).reshape(
                    n_cores, *out_avals[i].shape
                )[c]
                for i, name in enumerate(out_names)
            }
            for c in range(n_cores)
        ]

    return run


def _get_runner(nc, n_cores=_NCORES):
    key = id(nc)
    if key not in _RUNNERS:
        _RUNNERS[key] = _make_runner(nc, n_cores)
    return _RUNNERS[key]


def kernel(prediction, ground_truth):
    prediction = np.asarray(prediction, dtype=np.float32)
    ground_truth = np.asarray(ground_truth, dtype=np.float32)

    nc = _get_nc()
    in_maps = _prep_core_inputs(prediction, ground_truth)
    results = _get_runner(nc)(in_maps)

    out = np.zeros(_B, dtype=np.float32)
    for b in range(_B):
        dx = 0.0
        cms = []
        for h in range(2):
            r = results[2 * b + h]
            # rowparts[p, rb*G + g] = min over group g of row rb*128+p
            rp = r["rowparts"].reshape(128, _RB, _G).min(axis=2)  # [128, RB]
            dx += np.maximum(rp, 0.0).sum(dtype=np.float64)
            # colmin[p, j] = min over this core's row-blocks (partition p)
            cms.append(r["colmin"].astype(np.float32).min(axis=0))  # [N]
        cm = np.minimum(cms[0], cms[1])
        dy = np.maximum(cm, 0.0).sum(dtype=np.float64)
        out[b] = dx / _N + dy / _N
    return out


# revision 10
# speedup vs baseline: 19.0487x; 19.0487x over previous
"""Chamfer distance (CDLoss) Trainium2 kernel.

Problem: prediction [4, 8192, 3], ground_truth [4, 8192, 3] (fp32).
For each batch: d2[n,m] = max(||p_n||^2 + ||g_m||^2 - 2 p.g, 0);
out[b] = sum_n min_m d2 / N + sum_m min_n d2 / M.

Strategy (8 NeuronCores): core c handles (batch = c//2, row-half = c%2),
i.e. a 4096 x 8192 slab of the distance matrix.

Device kernel per core:
  - Augmented-coordinate trick: ap[5, 4096] = [px, py, pz, ||p||^2, 1],
    ag[5, 8192] = [-2gx, -2gy, -2gz, 1, ||g||^2] so a single K=5 fp32
    matmul tile emits squared distances directly into PSUM.
  - Tiles [128 rows x 2048 cols] (4 matmuls of N=512 into 4 PSUM banks).
  - One tensor_tensor_reduce per tile: copies PSUM -> SBUF (bf16) and
    min-reduces each row into a per-(rowblock, colgroup) partial (fp32,
    exact).
  - One bf16 tensor_tensor(min) per tile: running column-min buffer
    [128, 8192] (bf16 is exact-monotone: min of rounded = rounded min).
Host: final tiny reductions (min over 128 partitions / 4 groups, relu,
sums) in numpy.
"""

import numpy as np

_B = 4
_N = 8192  # points per cloud
_HALF = _N // 2  # rows per core
_RB = _HALF // 128  # 32 row blocks
_GW = 512  # column group width (one PSUM bank)
_G = _N // _GW  # 16 column groups
_NCORES = 8

_CACHED_NC = None
_RUNNERS = {}


def _build_nc(repeat=1):
    import concourse.bacc as bacc
    import concourse.tile as tile
    from concourse import mybir

    f32 = mybir.dt.float32
    bf16 = mybir.dt.bfloat16

    nc = bacc.Bacc("TRN2", target_bir_lowering=False, debug=False)

    ap_d = nc.dram_tensor("ap", [5, _HALF], f32, kind="ExternalInput")
    ag_d = nc.dram_tensor("ag", [5, _N], f32, kind="ExternalInput")
    rowparts_d = nc.dram_tensor(
        "rowparts", [128, _RB * _G], f32, kind="ExternalOutput"
    )
    colmin_d = nc.dram_tensor("colmin", [128, _N], bf16, kind="ExternalOutput")

    BIG = 1.0e38

    with tile.TileContext(nc) as tc:
        with (
            tc.tile_pool(name="singles", bufs=1) as singles,
            tc.tile_pool(name="spool", bufs=6) as spool,
            tc.tile_pool(name="psum", bufs=8, space="PSUM") as pp,
        ):
            ap_s = singles.tile([5, _HALF], f32)
            nc.sync.dma_start(out=ap_s[:], in_=ap_d[:])
            ag_s = singles.tile([5, _N], f32)
            nc.sync.dma_start(out=ag_s[:], in_=ag_d[:])

            colmin_s = singles.tile([128, _N], bf16)
            nc.vector.memset(colmin_s[:], BIG)
            rowparts_s = singles.tile([128, _RB * _G], f32)

            def _body():
                for rb in range(_RB):
                    lhsT = ap_s[:, rb * 128 : (rb + 1) * 128]
                    for g in range(_G):
                        t = pp.tile([128, _GW], f32, tag="t")
                        nc.tensor.matmul(
                            t[:],
                            lhsT,
                            ag_s[:, g * _GW : (g + 1) * _GW],
                            start=True,
                            stop=True,
                        )
                        idx = rb * _G + g
                        # exact fp32 row minima for this tile (DVE, PSUM src)
                        nc.vector.tensor_reduce(
                            rowparts_s[:, idx : idx + 1],
                            t[:],
                            axis=mybir.AxisListType.X,
                            op=mybir.AluOpType.min,
                        )
                        # PSUM -> SBUF exit on ScalarE, cast to bf16
                        s = spool.tile([128, _GW], bf16, tag="s")
                        nc.scalar.copy(s[:], t[:])
                        # running column minima (bf16, DVE 2x mode)
                        cslice = colmin_s[:, g * _GW : (g + 1) * _GW]
                        nc.vector.tensor_tensor(
                            cslice, cslice, s[:], op=mybir.AluOpType.min
                        )

            if repeat == 1:
                _body()
            else:
                # benchmark mode: body is idempotent (mins), repeat on-device
                with tc.For_i(0, repeat, 1):
                    _body()

            nc.sync.dma_start(out=rowparts_d[:], in_=rowparts_s[:])
            nc.sync.dma_start(out=colmin_d[:], in_=colmin_s[:])

    nc.compile()
    return nc


def _get_nc():
    global _CACHED_NC
    if _CACHED_NC is None:
        _CACHED_NC = _build_nc()
    return _CACHED_NC


def _prep_core_inputs(prediction, ground_truth):
    """Build per-core augmented matrices (host-side, fp32)."""
    in_maps = []
    for c in range(_NCORES):
        b, h = divmod(c, 2)
        p = np.asarray(prediction[b, h * _HALF : (h + 1) * _HALF], dtype=np.float32)
        g = np.asarray(ground_truth[b], dtype=np.float32)
        ap = np.empty((5, _HALF), dtype=np.float32)
        ap[0:3] = p.T
        ap[3] = (p * p).sum(axis=1, dtype=np.float32)
        ap[4] = 1.0
        ag = np.empty((5, _N), dtype=np.float32)
        ag[0:3] = (-2.0 * g).T
        ag[3] = 1.0
        ag[4] = (g * g).sum(axis=1, dtype=np.float32)
        in_maps.append({"ap": ap, "ag": ag})
    return in_maps


def _make_runner(nc, n_cores):
    """Build a cached jitted SPMD executor for `nc` (axon/PJRT path).

    Mirrors concourse.bass2jax.run_bass_via_pjrt but caches the jitted
    callable so repeat calls don't re-trace/re-compile.
    """
    import jax
    import numpy as _np
    from jax.sharding import Mesh, PartitionSpec
    from jax.experimental.shard_map import shard_map
    from concourse import mybir
    from concourse.bass2jax import (
        _bass_exec_p,
        install_neuronx_cc_hook,
        partition_id_tensor,
    )

    install_neuronx_cc_hook()

    partition_name = (
        nc.partition_id_tensor.name if nc.partition_id_tensor else None
    )
    in_names, out_names, out_avals, zero_shapes = [], [], [], []
    for alloc in nc.m.functions[0].allocations:
        if not isinstance(alloc, mybir.MemoryLocationSet):
            continue
        name = alloc.memorylocations[0].name
        if alloc.kind == "ExternalInput":
            if name == partition_name:
                continue
            in_names.append(name)
        elif alloc.kind == "ExternalOutput":
            shape = tuple(alloc.tensor_shape)
            dtype = mybir.dt.np(alloc.dtype)
            out_names.append(name)
            out_avals.append(jax.core.ShapedArray(shape, dtype))
            zero_shapes.append((shape, dtype))
    n_params = len(in_names)
    n_outs = len(out_names)
    all_names = in_names + out_names
    if partition_name is not None:
        all_names = all_names + [partition_name]
    donate = tuple(range(n_params, n_params + n_outs))

    def _body(*args):
        operands = list(args)
        if partition_name is not None:
            operands.append(partition_id_tensor())
        outs = _bass_exec_p.bind(
            *operands,
            out_avals=tuple(out_avals),
            in_names=tuple(all_names),
            out_names=tuple(out_names),
            lowering_input_output_aliases=(),
            sim_require_finite=True,
            sim_require_nnan=True,
            nc=nc,
        )
        return tuple(outs)

    devices = jax.devices()[:n_cores]
    mesh = Mesh(_np.asarray(devices), ("core",))
    sharded = jax.jit(
        shard_map(
            _body,
            mesh=mesh,
            in_specs=(PartitionSpec("core"),) * (n_params + n_outs),
            out_specs=(PartitionSpec("core"),) * n_outs,
            check_rep=False,
        ),
        donate_argnums=donate,
        keep_unused=True,
    )

    def run(in_maps):
        concat_in = [
            _np.concatenate([m[name] for m in in_maps], axis=0)
            for name in in_names
        ]
        concat_zeros = [
            _np.zeros((n_cores * s[0], *s[1:]), d) for (s, d) in zero_shapes
        ]
        out_arrs = sharded(*concat_in, *concat_zeros)
        return [
            {
                name: _np.asarray(out_arrs[i]).reshape(
                    n_cores, *out_avals[i].shape
                )[c]
                for i, name in enumerate(out_names)
            }
            for c in range(n_cores)
        ]

    return run


def _get_runner(nc, n_cores=_NCORES):
    key = id(nc)
    if key not in _RUNNERS:
        _RUNNERS[key] = _make_runner(nc, n_cores)
    return _RUNNERS[key]


def kernel(prediction, ground_truth):
    prediction = np.asarray(prediction, dtype=np.float32)
    ground_truth = np.asarray(ground_truth, dtype=np.float32)

    nc = _get_nc()
    in_maps = _prep_core_inputs(prediction, ground_truth)
    results = _get_runner(nc)(in_maps)

    out = np.zeros(_B, dtype=np.float32)
    for b in range(_B):
        dx = 0.0
        cms = []
        for h in range(2):
            r = results[2 * b + h]
            # rowparts[p, rb*G + g] = min over group g of row rb*128+p
            rp = r["rowparts"].reshape(128, _RB, _G).min(axis=2)  # [128, RB]
            dx += np.maximum(rp, 0.0).sum(dtype=np.float64)
            # colmin[p, j] = min over this core's row-blocks (partition p)
            cms.append(r["colmin"].astype(np.float32).min(axis=0))  # [N]
        cm = np.minimum(cms[0], cms[1])
        dy = np.maximum(cm, 0.0).sum(dtype=np.float64)
        out[b] = dx / _N + dy / _N
    return out


# revision 17
# speedup vs baseline: 26.6958x; 1.4014x over previous
"""Chamfer distance (CDLoss) Trainium2 kernel.

Problem: prediction [4, 8192, 3], ground_truth [4, 8192, 3] (fp32).
For each batch: d2[n,m] = max(||p_n||^2 + ||g_m||^2 - 2 p.g, 0);
out[b] = sum_n min_m d2 / N + sum_m min_n d2 / M.

Strategy (8 NeuronCores): core c handles (batch = c//2, row-half = c%2),
i.e. a 4096 x 8192 slab of the distance matrix.

Device kernel per core (32 row blocks x 16 column tiles of [128, 512]):
  - Augmented-coordinate trick: ap[5, 4096] = [px, py, pz, ||p||^2, 1],
    ag[5, 8192] = [-2gx, -2gy, -2gz, 1, ||g||^2] so a single K=5 fp32
    matmul emits a [128, 512] tile of squared distances into one PSUM
    bank (PE time ~N cycles regardless of K).
  - VectorE tensor_reduce(min) per tile: exact fp32 row-min partial per
    (rowblock, coltile) into rowparts[128, 32*16].
  - ScalarE copy: PSUM -> SBUF cast to bf16 (the only other PSUM exit).
  - VectorE tensor_tensor(min) in bf16 (2x perf mode): running
    column-min buffer [128, 8192]. bf16 min is exact-monotone
    (min of rounded = rounded min), and the final sum of 8192 values
    concentrates the rounding to ~1e-5 relative.
Host: final tiny reductions (min over 128 partitions / 16 col tiles,
relu clamp, sums) in numpy. min-then-clamp == clamp-then-min, so the
relu of the reference moves to the host gather.
"""

import numpy as np

_B = 4
_N = 8192  # points per cloud
_HALF = _N // 2  # rows per core
_RB = _HALF // 128  # 32 row blocks
_GW = 512  # column group width (one PSUM bank)
_G = _N // _GW  # 16 column groups
_NCORES = 8

_CACHED_NC = None
_RUNNERS = {}


def _build_nc(repeat=1, gpsimd_share=0.0):
    import concourse.bacc as bacc
    import concourse.tile as tile
    from concourse import mybir

    f32 = mybir.dt.float32
    bf16 = mybir.dt.bfloat16

    nc = bacc.Bacc("TRN2", target_bir_lowering=False, debug=False)

    ap_d = nc.dram_tensor("ap", [5, _HALF], f32, kind="ExternalInput")
    ag_d = nc.dram_tensor("ag", [5, _N], f32, kind="ExternalInput")
    rowparts_d = nc.dram_tensor(
        "rowparts", [128, _RB * _G], f32, kind="ExternalOutput"
    )
    colmin_d = nc.dram_tensor("colmin", [128, _N], bf16, kind="ExternalOutput")

    BIG = 1.0e38

    with tile.TileContext(nc) as tc:
        with (
            tc.tile_pool(name="singles", bufs=1) as singles,
            tc.tile_pool(name="spool", bufs=6) as spool,
            tc.tile_pool(name="psum", bufs=8, space="PSUM") as pp,
        ):
            ap_s = singles.tile([5, _HALF], f32)
            nc.sync.dma_start(out=ap_s[:], in_=ap_d[:])
            ag_s = singles.tile([5, _N], f32)
            nc.sync.dma_start(out=ag_s[:], in_=ag_d[:])

            colmin_s = singles.tile([128, _N], bf16)
            nc.vector.memset(colmin_s[:], BIG)
            rowparts_s = singles.tile([128, _RB * _G], f32)

            def _body():
                for rb in range(_RB):
                    lhsT = ap_s[:, rb * 128 : (rb + 1) * 128]
                    for g in range(_G):
                        t = pp.tile([128, _GW], f32, tag="t")
                        nc.tensor.matmul(
                            t[:],
                            lhsT,
                            ag_s[:, g * _GW : (g + 1) * _GW],
                            start=True,
                            stop=True,
                        )
                        idx = rb * _G + g
                        # exact fp32 row minima for this tile (DVE, PSUM src)
                        nc.vector.tensor_reduce(
                            rowparts_s[:, idx : idx + 1],
                            t[:],
                            axis=mybir.AxisListType.X,
                            op=mybir.AluOpType.min,
                        )
                        # PSUM -> SBUF exit on ScalarE, cast to bf16
                        s = spool.tile([128, _GW], bf16, tag="s")
                        nc.scalar.copy(s[:], t[:])
                        # running column minima (bf16, DVE 2x mode)
                        cslice = colmin_s[:, g * _GW : (g + 1) * _GW]
                        nc.vector.tensor_tensor(
                            cslice, cslice, s[:], op=mybir.AluOpType.min
                        )

            if repeat == 1:
                _body()
            else:
                # benchmark mode: body is idempotent (mins), repeat on-device
                with tc.For_i(0, repeat, 1):
                    _body()

            nc.sync.dma_start(out=rowparts_d[:], in_=rowparts_s[:])
            nc.sync.dma_start(out=colmin_d[:], in_=colmin_s[:])

    nc.compile()
    return nc


def _get_nc():
    global _CACHED_NC
    if _CACHED_NC is None:
        _CACHED_NC = _build_nc()
    return _CACHED_NC


def _prep_core_inputs(prediction, ground_truth):
    """Build per-core augmented matrices (host-side, fp32)."""
    in_maps = []
    for c in range(_NCORES):
        b, h = divmod(c, 2)
        p = np.asarray(prediction[b, h * _HALF : (h + 1) * _HALF], dtype=np.float32)
        g = np.asarray(ground_truth[b], dtype=np.float32)
        ap = np.empty((5, _HALF), dtype=np.float32)
        ap[0:3] = p.T
        ap[3] = (p * p).sum(axis=1, dtype=np.float32)
        ap[4] = 1.0
        ag = np.empty((5, _N), dtype=np.float32)
        ag[0:3] = (-2.0 * g).T
        ag[3] = 1.0
        ag[4] = (g * g).sum(axis=1, dtype=np.float32)
        in_maps.append({"ap": ap, "ag": ag})
    return in_maps


def _make_runner(nc, n_cores):
    """Build a cached jitted SPMD executor for `nc` (axon/PJRT path).

    Mirrors concourse.bass2jax.run_bass_via_pjrt but caches the jitted
    callable so repeat calls don't re-trace/re-compile.
    """
    import jax
    import numpy as _np
    from jax.sharding import Mesh, PartitionSpec
    from jax.experimental.shard_map import shard_map
    from concourse import mybir
    from concourse.bass2jax import (
        _bass_exec_p,
        install_neuronx_cc_hook,
        partition_id_tensor,
    )

    install_neuronx_cc_hook()

    partition_name = (
        nc.partition_id_tensor.name if nc.partition_id_tensor else None
    )
    in_names, out_names, out_avals, zero_shapes = [], [], [], []
    for alloc in nc.m.functions[0].allocations:
        if not isinstance(alloc, mybir.MemoryLocationSet):
            continue
        name = alloc.memorylocations[0].name
        if alloc.kind == "ExternalInput":
            if name == partition_name:
                continue
            in_names.append(name)
        elif alloc.kind == "ExternalOutput":
            shape = tuple(alloc.tensor_shape)
            dtype = mybir.dt.np(alloc.dtype)
            out_names.append(name)
            out_avals.append(jax.core.ShapedArray(shape, dtype))
            zero_shapes.append((shape, dtype))
    n_params = len(in_names)
    n_outs = len(out_names)
    all_names = in_names + out_names
    if partition_name is not None:
        all_names = all_names + [partition_name]
    donate = tuple(range(n_params, n_params + n_outs))

    def _body(*args):
        operands = list(args)
        if partition_name is not None:
            operands.append(partition_id_tensor())
        outs = _bass_exec_p.bind(
            *operands,
            out_avals=tuple(out_avals),
            in_names=tuple(all_names),
            out_names=tuple(out_names),
            lowering_input_output_aliases=(),
            sim_require_finite=True,
            sim_require_nnan=True,
            nc=nc,
        )
        return tuple(outs)

    devices = jax.devices()[:n_cores]
    mesh = Mesh(_np.asarray(devices), ("core",))
    sharded = jax.jit(
        shard_map(
            _body,
            mesh=mesh,
            in_specs=(PartitionSpec("core"),) * (n_params + n_outs),
            out_specs=(PartitionSpec("core"),) * n_outs,
            check_rep=False,
        ),
        donate_argnums=donate,
        keep_unused=True,
    )

    def run(in_maps):
        concat_in = [
            _np.concatenate([m[name] for m in in_maps], axis=0)
            for name in in_names
        ]
        concat_zeros = [
            _np.zeros((n_cores * s[0], *s[1:]), d) for (s, d) in zero_shapes
        ]
        out_arrs = sharded(*concat_in, *concat_zeros)
        return [
            {
                name: _np.asarray(out_arrs[i]).reshape(
                    n_cores, *out_avals[i].shape
                )[c]
                for i, name in enumerate(out_names)
            }
            for c in range(n_cores)
        ]

    return run


def _get_runner(nc, n_cores=_NCORES):
    key = id(nc)
    if key not in _RUNNERS:
        _RUNNERS[key] = _make_runner(nc, n_cores)
    return _RUNNERS[key]


def kernel(prediction, ground_truth):
    prediction = np.asarray(prediction, dtype=np.float32)
    ground_truth = np.asarray(ground_truth, dtype=np.float32)

    nc = _get_nc()
    in_maps = _prep_core_inputs(prediction, ground_truth)
    results = _get_runner(nc)(in_maps)

    out = np.zeros(_B, dtype=np.float32)
    for b in range(_B):
        dx = 0.0
        cms = []
        for h in range(2):
            r = results[2 * b + h]
            # rowparts[p, rb*G + g] = min over group g of row rb*128+p
            rp = r["rowparts"].reshape(128, _RB, _G).min(axis=2)  # [128, RB]
            dx += np.maximum(rp, 0.0).sum(dtype=np.float64)
            # colmin[p, j] = min over this core's row-blocks (partition p)
            cms.append(r["colmin"].astype(np.float32).min(axis=0))  # [N]
        cm = np.minimum(cms[0], cms[1])
        dy = np.maximum(cm, 0.0).sum(dtype=np.float64)
        out[b] = dx / _N + dy / _N
    return out
